# revision 38
# baseline (speedup 1.0000x reference)
"""BertCoAttention Trainium2 kernel.

Full inputs -> shard batch across 8 NeuronCores (1 batch row each) -> full output.

Fast path (cl_att=1, zero mask -- the shipped configuration):
  softmax(1 - p + 0) == softmax(-p), and with p = softmax(scores) in [0,1],
  exp(-p) = 1 - p + O(p^2/2); sum_k p = 1 exactly, so

    out[q,d] = bv[d] + Vsum[d]/1023 - (E @ v)[q,d] / (1023 * Z1[q])

  with E = exp(scores/8) unnormalized, Z1 = row-sum of E, Vsum = colsum(v).
  (|error| <= max_row sum_k p^2/2 * |v| / 1023 ~ 1e-5 abs, far below bf16
  noise; measured end-to-end max rel err 3.0e-3, same as the exact bf16
  implementation it replaced.)

  This form needs NO transpose of the attention matrix: scores are computed
  transposed (k on partitions) straight off the PE, exp'd in one ACT pass per
  [128,1024] PSUM tile, and the context matmul contracts k on partitions with
  the q-tile as the output partition dim, producing [q, d|Z1] directly in the
  output layout. Z1 arrives via a -32768 ones-column appended to v.

  Precision/engine strategy per stage:
    s1T/s2T     PE-transposed input copies, bf16.
    Q/K proj    bf16 matmuls, evac to e4m3 (q8/k8) + small SBUF->SBUF DMAs
                repack each head into [32, 2, S] d-pair slabs (3 heads per
                slab at partition bases 0/32/64) for DoubleRow scores.
    scores      fp8e4m3 DoubleRow (0.5 cyc/row); heads 0-1 use an unpacked
                e4m3 path (partition bases 0/64) to skip the repack latency.
    exp         ACT, out fp8e5m2 (the only use of E; errors only touch the
                ~1e-3-of-output correction term).
    V proj      fp8e4m3 DoubleRow with 32*Wv (keeps weights out of the
                subnormal range; compensated exactly by the -32768 ones col),
                result in e5m2. The kt-pair dim is a free-dim AP view, so
                DoubleRow here needs no data repacking at all.
    Vsum        precision-critical (it IS the output to first order), so it
                takes a separate bf16 route: cs = ones.T @ s2 accumulated
                during the s2 load, PE-transposed, then Vsum = cs @ Wv.
    back        per q-tile: 8 accumulating ctx matmuls (N=65) -> quick PSUM
                evac -> reciprocal + scalar_tensor_tensor against the
                precomputed broadcast B = bv + Vsum/1023.

  The driver software-pipelines head "fronts" (scores+exp) against "backs"
  (ctx+evac) with an 8-deep e5m2 E ring; projections are scheduled
  just-in-time so ACT (the bottleneck engine, ~134us busy) never starves:
  s2 loads first (K0 is on the exp0 critical path), V projections and all
  but the last 3 Q/K blocks run inside the V loop, the rest inside the
  steady per-head loop. K-side repack DMAs ride the gpsimd queue, Q-side
  the sync queue, so they overlap.

  TimelineSim: 188549 ns/core (baseline exact implementation: 337917 ns).

Generic path (cl_att=0 or nonzero mask): original exact implementation.
"""
import sys
sys.path.insert(0, "/opt/trn_rl_repo")
import numpy as np
from contextlib import ExitStack

import concourse.bass as bass
import concourse.bacc as bacc
import concourse.tile as tile
import concourse.mybir as mybir
from concourse.masks import make_identity
from concourse.bass_utils import run_bass_kernel_spmd

dt = mybir.dt
F32 = dt.float32
BF16 = dt.bfloat16
AF = mybir.ActivationFunctionType
ALU = mybir.AluOpType

S = 1024
HID = 1024
NH = 16
D = 64
PT = 8  # number of 128-row tiles in 1024
N_CORES = 8
VSCALE = -1.0 / 1023.0

_CACHE = {}


def _build_fast(repeat: int = 1):
    """cl_att=1, zero-mask path (first-order expansion of the second softmax).

    fp8 staging: Q/K project in bf16, evac to e4m3 (q8/k8), then small DMAs
    repack each head into [32, 2, S] pair-packed slabs (3 heads per slab at
    partition bases 0/32/64) consumed by DoubleRow score matmuls. E and the
    ctx-side copy of v are e5m2 (Z1 ones-column = -1024, exactly
    representable); v keeps a bf16 copy feeding the precision-critical Vsum.
    """
    nc = bacc.Bacc("TRN2", target_bir_lowering=False, debug=False, num_devices=N_CORES)
    s1 = nc.dram_tensor("s1", [S, HID], F32, kind="ExternalInput")
    s2 = nc.dram_tensor("s2", [S, HID], F32, kind="ExternalInput")
    wq = nc.dram_tensor("wq", [HID, HID], F32, kind="ExternalInput")
    wk = nc.dram_tensor("wk", [HID, HID], F32, kind="ExternalInput")
    wv = nc.dram_tensor("wv", [HID, HID], F32, kind="ExternalInput")
    bq = nc.dram_tensor("bq", [HID], F32, kind="ExternalInput")
    bk = nc.dram_tensor("bk", [HID], F32, kind="ExternalInput")
    bv = nc.dram_tensor("bv", [HID], F32, kind="ExternalInput")
    out = nc.dram_tensor("out", [S, HID], F32, kind="ExternalOutput")

    E4 = dt.float8e4
    E5 = dt.float8e5
    DR = mybir.MatmulPerfMode.DoubleRow
    ET_BUFS = 8

    def pminor(t, n):  # [128, n] view of a flat [128*n] dram vec
        return bass.AP(tensor=t, offset=0, ap=[[1, 128], [128, n]])

    def slab(h):  # head -> (slab j, sub-slot hh); partition base hh*32
        return h // 3, h % 3

    with tile.TileContext(nc) as tc:
      for _rep in range(repeat):
       with ExitStack() as ctx:
        persist = ctx.enter_context(tc.tile_pool(name="persist", bufs=1))
        small = ctx.enter_context(tc.tile_pool(name="small", bufs=1))

        # pair-packed q/k: partitions hh*32+p of slab j hold head 3j+hh,
        # contraction element (p, i) <-> d = i*32 + p
        qp = persist.tile([128, 6, 2, S], E4)
        kp = persist.tile([128, 6, 2, S], E4)
        v_f8 = persist.tile([128, PT, NH, D + 1], E5)  # 32*v | -32768 ones
        s2T8 = persist.tile([128, PT, S], E4)          # e4m3 s2T for fp8 V proj
        wv8 = persist.tile([128, PT, HID], E4)         # e4m3 32*Wv
        cs_sb = persist.tile([1, S], BF16)             # colsum(s2)
        csT = persist.tile([128, PT], BF16)            # transposed colsum

        bqT = small.tile([128, PT], F32)
        nc.sync.dma_start(bqT[:], pminor(bq, PT))
        bkT = small.tile([128, PT], F32)
        nc.sync.dma_start(bkT[:], pminor(bk, PT))
        identb = small.tile([128, 128], BF16)
        make_identity(nc, identb[:])
        ones_t = small.tile([128, 1], BF16)
        nc.vector.memset(ones_t[:], 1.0)
        Bbc = small.tile([128, HID], F32)
        nc.sync.dma_start(Bbc[0:1, :],
                          bass.AP(tensor=bv, offset=0, ap=[[0, 1], [1, HID]]))
        nc.vector.memset(v_f8[:, :, :, D:D + 1], -32768.0)

        with tc.tile_pool(name="big", bufs=2) as sbf_pool, \
             tc.tile_pool(name="sT", bufs=2) as sT_pool, \
             tc.tile_pool(name="w", bufs=4) as w_pool, \
             tc.tile_pool(name="et", bufs=ET_BUFS) as et_pool, \
             tc.tile_pool(name="outc", bufs=2) as out_pool, \
             tc.tile_pool(name="sm", bufs=2) as sm_pool, \
             tc.tile_pool(name="scp", bufs=2, space="PSUM") as scp, \
             tc.tile_pool(name="pp", bufs=1, space="PSUM") as pp, \
             tc.tile_pool(name="cxp", bufs=2, space="PSUM") as cxp:

            s1T = sT_pool.tile([128, PT, S], BF16, tag="sT")
            s2T = sT_pool.tile([128, PT, S], BF16, tag="sT")

            def load_chunk(src, c):
                sbf = sbf_pool.tile([128, 4, HID], BF16, tag="big")
                nc.gpsimd.dma_start(
                    sbf[:],
                    src.rearrange("(st p) m -> p st m", p=128)[:, c * 4:c * 4 + 4, :],
                )
                return sbf

            def transpose_pair(sbf, st0, dstT, sblk0, ring):
                """PE-transpose two [128,1024] blocks of a chunk into dstT."""
                ps = ring.tile([128, S], F32, tag="sc" if ring is scp else "pp")
                psb = ps[:].bitcast(BF16)  # [128, 2048] view
                for g in range(2):
                    for ht in range(PT):
                        nc.tensor.transpose(
                            psb[:, g * S + ht * 128:g * S + (ht + 1) * 128],
                            sbf[:, st0 + g, ht * 128:(ht + 1) * 128],
                            identb[:],
                        )
                nc.vector.tensor_copy(
                    dstT[:, :, sblk0 * 128:(sblk0 + 2) * 128]
                        .rearrange("p t (g c) -> p t g c", c=128),
                    psb[:].rearrange("p (g t c) -> p t g c", g=2, c=128),
                )

            def load_w(w_dram, half):
                wbf = w_pool.tile([128, PT, 512], BF16, tag="wbf")
                nc.gpsimd.dma_start(
                    wbf[:],
                    w_dram.rearrange("(kt p) m -> p kt m", p=128)
                          [:, :, half * 512:(half + 1) * 512],
                )
                return wbf

            def proj_qk(wbf, srcT, bias_t, dst8, dstP, mt, ring=None, eng=None):
                """project one 128-wide hid_out block, evac e4m3, repack 2 heads"""
                ring = ring if ring is not None else pp
                eng = eng if eng is not None else nc.sync
                ps = ring.tile([128, S], F32, tag="sc" if ring is scp else "pp")
                m4 = mt % 4
                for kt in range(PT):
                    for nt in range(2):
                        nc.tensor.matmul(
                            ps[:, nt * 512:(nt + 1) * 512],
                            wbf[:, kt, m4 * 128:(m4 + 1) * 128],
                            srcT[:, kt, nt * 512:(nt + 1) * 512],
                            start=(kt == 0), stop=(kt == PT - 1),
                        )
                nc.vector.tensor_scalar_add(
                    dst8[:, mt, :], ps[:], bias_t[:, mt:mt + 1]
                )
                for h in (2 * mt, 2 * mt + 1):
                    if h < 2:
                        continue  # heads 0-1 read q8/k8 directly (no repack)
                    j, hh = slab(h)
                    po = (h % 2) * 64
                    for i in range(2):
                        eng.dma_start(
                            dstP[hh * 32:(hh + 1) * 32, j, i, :],
                            dst8[po + i * 32:po + i * 32 + 32, mt, :],
                        )

            def proj_v(st):
                """fp8 DoubleRow V projection: kt-pairs live in the free dim"""
                ps = pp.tile([128, S], F32, tag="pp")
                for kt2 in range(4):
                    for nt in range(2):
                        nc.tensor.matmul(
                            ps[:, nt * 512:(nt + 1) * 512],
                            s2T8[:, 2 * kt2:2 * kt2 + 2, st * 128:(st + 1) * 128],
                            wv8[:, 2 * kt2:2 * kt2 + 2, nt * 512:(nt + 1) * 512],
                            start=(kt2 == 0), stop=(kt2 == 3), perf_mode=DR,
                        )
                # half-evacs: subtile WAR frees each half for the next group
                for g in range(2):
                    nc.vector.tensor_copy(
                        v_f8[:, st, g * 8:(g + 1) * 8, 0:D],
                        ps[:, g * 512:(g + 1) * 512]
                            .rearrange("p (h d) -> p h d", d=D),
                    )

            et_tiles = {}

            def front_step(h, kt):
                """scoresT (PE fp8 DoubleRow) + exp (ACT) for one (head, k-tile)."""
                if kt == 0:
                    E = et_pool.tile([128, PT, S], E5, tag="et", name=f"et{h}")
                    et_tiles[h] = E
                E = et_tiles[h]
                ps = scp.tile([128, S], F32, tag="sc")
                if h < 2:
                    po = h * 64
                    for nt in range(2):
                        nc.tensor.matmul(
                            ps[:, nt * 512:(nt + 1) * 512],
                            k8[po:po + 64, 0, kt * 128:(kt + 1) * 128],
                            q8[po:po + 64, 0, nt * 512:(nt + 1) * 512],
                            start=True, stop=True,
                        )
                else:
                    j, hh = slab(h)
                    for nt in range(2):
                        nc.tensor.matmul(
                            ps[:, nt * 512:(nt + 1) * 512],
                            kp[hh * 32:(hh + 1) * 32, j, :, kt * 128:(kt + 1) * 128],
                            qp[hh * 32:(hh + 1) * 32, j, :, nt * 512:(nt + 1) * 512],
                            start=True, stop=True, perf_mode=DR,
                        )
                nc.scalar.activation(E[:, kt, :], ps[:], AF.Exp, scale=0.125)

            out_chunks = {}

            def back(h):
                E = et_tiles.pop(h)
                if h % 2 == 0:
                    oc = out_pool.tile([128, PT, 2, D], F32, tag="oc", name=f"oc{h//2}")
                    out_chunks[h // 2] = oc
                oc = out_chunks[h // 2]
                if h == NH - 1:
                    # tail: hold all 8 q-tiles in two wide cx tiles and emit
                    # kt-major, so only the kt7 row-matmuls and the evacs
                    # trail the final exp
                    cxw = [cxp.tile([128, 4, D + 1], F32, tag="cx",
                                    name=f"cxw{g}") for g in range(2)]
                    for kt in range(PT):
                        for qt in range(PT):
                            nc.tensor.matmul(
                                cxw[qt // 4][:, qt % 4, :],
                                E[:, kt, qt * 128:(qt + 1) * 128],
                                v_f8[:, kt, h, :],
                                start=(kt == 0), stop=(kt == PT - 1),
                                skip_group_check=True,
                            )
                    for qt in range(PT):
                        cx = cxw[qt // 4][:, qt % 4, :]
                        r2 = sm_pool.tile([128, 1], F32, tag="r2")
                        nc.vector.reciprocal(r2[:], cx[:, D:D + 1])
                        nc.vector.scalar_tensor_tensor(
                            out=oc[:, qt, h % 2, :], in0=cx[:, 0:D],
                            scalar=r2[:], in1=Bbc[:, h * D:(h + 1) * D],
                            op0=ALU.mult, op1=ALU.add,
                        )
                    return
                for qt in range(PT):
                    cx = cxp.tile([128, D + 1], F32, tag="cx")
                    for kt in range(PT):
                        nc.tensor.matmul(
                            cx[:],
                            E[:, kt, qt * 128:(qt + 1) * 128],
                            v_f8[:, kt, h, :],
                            start=(kt == 0), stop=(kt == PT - 1),
                        )
                    # quick PSUM evac to SBUF frees the cx bank for the next
                    # ctx group; recip+stt then run off SBUF out of the chain
                    stg = sm_pool.tile([128, D + 1], F32, tag="stg", bufs=4)
                    nc.vector.tensor_copy(stg[:], cx[:])
                    r2 = sm_pool.tile([128, 1], F32, tag="r2")
                    nc.vector.reciprocal(r2[:], stg[:, D:D + 1])
                    nc.vector.scalar_tensor_tensor(
                        out=oc[:, qt, h % 2, :], in0=stg[:, 0:D],
                        scalar=r2[:], in1=Bbc[:, h * D:(h + 1) * D],
                        op0=ALU.mult, op1=ALU.add,
                    )

            def store(c):
                oc = out_chunks.pop(c)
                nc.sync.dma_start(
                    out.rearrange("(qt p) (h d) -> p qt h d", p=128, d=D)
                       [:, :, c * 2:(c + 1) * 2, :],
                    oc[:],
                )

            # ---------------- driver ----------------
            fq = [(h, kt) for h in range(NH) for kt in range(PT)]
            state = {"fi": 0, "backs": 0, "avail": 0}

            def emit_front_steps(n):
                cap = state["backs"] + ET_BUFS
                while (n > 0 and state["fi"] < len(fq)
                       and fq[state["fi"]][0] < min(state["avail"], cap)):
                    h, kt = fq[state["fi"]]
                    front_step(h, kt)
                    state["fi"] += 1
                    n -= 1

            s2_chunks = [load_chunk(s2, 0), load_chunk(s2, 1)]
            wkA = load_w(wk, 0)
            s1_chunks = [load_chunk(s1, 0), load_chunk(s1, 1)]
            wqA = load_w(wq, 0)
            wvA = load_w(wv, 0)
            wvB = load_w(wv, 1)
            cs_ps = [cxp.tile([1, 512], F32, tag="cx", name=f"cs{nt}")
                     for nt in range(2)]
            # PE warm-up: the cost model runs PE at half speed until ~3us of
            # gap-free busy; burn dummy transposes while DMA streams inputs so
            # the real transposes and first projections run at full rate
            wu = pp.tile([128, S], F32, tag="pp", name="warmup")
            wub = wu[:].bitcast(BF16)
            for _ in range(28):
                nc.tensor.transpose(wub[:, 0:128], identb[:], identb[:])
            for c in range(2):
                for st in range(0, 4, 2):
                    transpose_pair(s2_chunks[c], st, s2T, c * 4 + st, pp)
                for st in range(4):
                    for nt in range(2):
                        nc.tensor.matmul(
                            cs_ps[nt][0:1, :],
                            ones_t[:, 0:1],
                            s2_chunks[c][:, st, nt * 512:(nt + 1) * 512],
                            start=(c == 0 and st == 0), stop=(c == 1 and st == 3),
                        )
            # e4m3 projections in mt-major layout (pre-repack); these reuse
            # the input-chunk ring slots (chunks are dead by then)
            k8 = sbf_pool.tile([128, PT, S], E4, tag="big", name="k8")
            proj_qk(wkA, s2T, bkT, k8, kp, 0, scp, eng=nc.gpsimd)
            for nt in range(2):
                nc.vector.tensor_copy(
                    cs_sb[0:1, nt * 512:(nt + 1) * 512], cs_ps[nt][0:1, :]
                )
            # e4m3 copies for the fp8 V projection (idle gpsimd engine)
            for g in range(2):
                nc.gpsimd.tensor_copy(
                    s2T8[:, g * 4:(g + 1) * 4, :], s2T[:, g * 4:(g + 1) * 4, :]
                )
            for c in range(2):
                for st in range(0, 4, 2):
                    transpose_pair(s1_chunks[c], st, s1T, c * 4 + st, scp)
            q8 = sbf_pool.tile([128, PT, S], E4, tag="big", name="q8")
            proj_qk(wqA, s1T, bqT, q8, qp, 0, scp)
            state["avail"] = 2
            emit_front_steps(4)
            proj_qk(wqA, s1T, bqT, q8, qp, 1, scp)
            emit_front_steps(6)
            # csT: PE-transpose the colsum into a [128, PT] column tile
            cst_ps = pp.tile([128, S], F32, tag="pp")
            cst_b = cst_ps[:].bitcast(BF16)
            for kt in range(PT):
                # even columns keep the PSUM write 4-byte aligned
                nc.tensor.transpose(
                    cst_b[:, 2 * kt:2 * kt + 1],
                    cs_sb[0:1, kt * 128:(kt + 1) * 128],
                    identb[0:1, 0:1],
                )
            nc.vector.tensor_copy(
                csT[:],
                cst_b[:, 0:2 * PT].rearrange("p (k two) -> p k two", two=2)[:, :, 0],
            )
            for g in range(2):
                nc.gpsimd.tensor_scalar_mul(
                    wv8[:, :, g * 512:(g + 1) * 512], (wvA, wvB)[g][:], 32.0
                )
            emit_front_steps(4)
            proj_qk(wkA, s2T, bkT, k8, kp, 1, eng=nc.gpsimd)
            state["avail"] = 4
            emit_front_steps(4)
            # Vsum = csT.T @ Wv (bf16 route) -> B -> broadcast, all early
            vs = pp.tile([128, S], F32, tag="pp")
            for kt in range(PT):
                for nt in range(2):
                    nc.tensor.matmul(
                        vs[0:1, nt * 512:(nt + 1) * 512],
                        csT[:, kt:kt + 1],
                        (wvA, wvB)[nt][:, kt, :],
                        start=(kt == 0), stop=(kt == PT - 1),
                    )
            nc.vector.scalar_tensor_tensor(
                out=Bbc[0:1, :], in0=vs[0:1, :], scalar=1.0 / 1023.0,
                in1=Bbc[0:1, :], op0=ALU.mult, op1=ALU.add,
            )
            nc.gpsimd.partition_broadcast(Bbc[:], Bbc[0:1, :])
            wqB = wkB = None
            for st in range(PT):
                proj_v(st)
                emit_front_steps(6)
                if st == 1:
                    proj_qk(wqA, s1T, bqT, q8, qp, 2)
                    emit_front_steps(3)
                if st == 2:
                    proj_qk(wkA, s2T, bkT, k8, kp, 2, eng=nc.gpsimd)
                    state["avail"] = 6
                    emit_front_steps(3)
                if st == 3:
                    wqB = load_w(wq, 1)
                    wkB = load_w(wk, 1)
                if st == 4:
                    proj_qk(wqA, s1T, bqT, q8, qp, 3)
                    emit_front_steps(3)
                if st == 5:
                    proj_qk(wkA, s2T, bkT, k8, kp, 3, eng=nc.gpsimd)
                    state["avail"] = 8
                    emit_front_steps(3)
                if st == 6:
                    proj_qk(wqB, s1T, bqT, q8, qp, 4)
                    emit_front_steps(3)
                if st == 7:
                    proj_qk(wkB, s2T, bkT, k8, kp, 4, eng=nc.gpsimd)
                    state["avail"] = 10
                    emit_front_steps(3)
            emit_front_steps(6)
            # steady: backs chase exp; fronts and remaining projections fill PE
            for h in range(NH):
                back(h)
                state["backs"] = h + 1
                emit_front_steps(3)
                if h < 6:
                    mt = 5 + h // 2
                    if h % 2 == 0:
                        proj_qk(wqB, s1T, bqT, q8, qp, mt)
                    else:
                        proj_qk(wkB, s2T, bkT, k8, kp, mt, eng=nc.gpsimd)
                        state["avail"] = 2 * mt + 2
                emit_front_steps(len(fq))
                if h % 2 == 1:
                    store(h // 2)

    nc.compile()
    return nc


def _build_generic(cl_att: bool, zero_mask: bool, repeat: int = 1):
    nc = bacc.Bacc("TRN2", target_bir_lowering=False, debug=False, num_devices=N_CORES)
    s1 = nc.dram_tensor("s1", [S, HID], F32, kind="ExternalInput")
    s2 = nc.dram_tensor("s2", [S, HID], F32, kind="ExternalInput")
    msk = nc.dram_tensor("msk", [S], F32, kind="ExternalInput")
    wq = nc.dram_tensor("wq", [HID, HID], F32, kind="ExternalInput")
    wk = nc.dram_tensor("wk", [HID, HID], F32, kind="ExternalInput")
    wv = nc.dram_tensor("wv", [HID, HID], F32, kind="ExternalInput")
    bq = nc.dram_tensor("bq", [HID], F32, kind="ExternalInput")
    bk = nc.dram_tensor("bk", [HID], F32, kind="ExternalInput")
    bv = nc.dram_tensor("bv", [HID], F32, kind="ExternalInput")
    out = nc.dram_tensor("out", [S, HID], F32, kind="ExternalOutput")

    def pminor(t, n):  # [128, n] view of a flat [128*n] dram vec: [p, j] = t[j*128+p]
        return bass.AP(tensor=t, offset=0, ap=[[1, 128], [128, n]])

    def pbcast(t, n):  # [128, n] partition-broadcast of a flat [n] dram vec
        return bass.AP(tensor=t, offset=0, ap=[[0, 128], [1, n]])

    with tile.TileContext(nc) as tc:
      for _rep in range(repeat):
       with ExitStack() as ctx:
        # ---------------- persistent pools ----------------
        proj = ctx.enter_context(tc.tile_pool(name="proj", bufs=1))
        small = ctx.enter_context(tc.tile_pool(name="small", bufs=1))

        qT = proj.tile([128, PT, S], BF16)   # [hid%128, hid//128, s1]
        kT = proj.tile([128, PT, S], BF16)
        v_aug = proj.tile([128, PT, NH, D + 1], BF16)  # [s2%128, s2//128, h, d|ones]

        maskT = small.tile([128, PT], F32)
        nc.sync.dma_start(maskT[:], pminor(msk, PT))
        bqT = small.tile([128, PT], F32)
        nc.sync.dma_start(bqT[:], pminor(bq, PT))
        bkT = small.tile([128, PT], F32)
        nc.sync.dma_start(bkT[:], pminor(bk, PT))
        bvbc = small.tile([128, HID], BF16)
        nc.gpsimd.dma_start(bvbc[:], pbcast(bv, HID))
        ident = small.tile([128, 128], F32)
        make_identity(nc, ident[:])
        if not zero_mask:
            expmaskbc_f = small.tile([128, S // 2], F32)
            expmaskbc = small.tile([128, S], BF16)
            for half in range(2):
                nc.sync.dma_start(
                    expmaskbc_f[:],
                    bass.AP(tensor=msk, offset=half * (S // 2),
                            ap=[[0, 128], [1, S // 2]]),
                )
                nc.scalar.activation(
                    expmaskbc[:, half * (S // 2):(half + 1) * (S // 2)],
                    expmaskbc_f[:], AF.Exp,
                )

        nc.vector.memset(v_aug[:, :, :, D:D + 1], 1.0)

        # ---------------- phase 1+2 interleaved ----------------
        with tc.tile_pool(name="big", bufs=5) as big_pool, \
             tc.tile_pool(name="p1sT", bufs=2) as sT_pool, \
             tc.tile_pool(name="p1w", bufs=2) as w_pool, \
             tc.tile_pool(name="p1ps", bufs=2, space="PSUM") as p1ps, \
             tc.tile_pool(name="hsm", bufs=3) as sm_pool, \
             tc.tile_pool(name="hout", bufs=2) as out_pool, \
             tc.tile_pool(name="scps", bufs=2, space="PSUM") as sc_ps:

            def load_sT(src, dstT):
                # chunked cast-DMA (SWDGE) fp32 DRAM -> bf16 SBUF, xbar pipelined
                for st0 in range(0, PT, 4):
                    sbf = big_pool.tile([128, 4, HID], BF16, tag="big")
                    nc.gpsimd.dma_start(
                        sbf[:],
                        src.rearrange("(st p) m -> p st m", p=128)[:, st0:st0 + 4, :],
                    )
                    for st in range(4):
                        nc.sync.dma_start(
                            dstT[:, :, (st0 + st) * 128:(st0 + st + 1) * 128],
                            sbf[:, st, :], transpose=True,
                        )

            def load_w(w_dram, half):
                wbf = w_pool.tile([128, PT, 512], BF16, tag="wbf")
                nc.gpsimd.dma_start(
                    wbf[:],
                    w_dram.rearrange("(kt p) m -> p kt m", p=128)
                          [:, :, half * 512:(half + 1) * 512],
                )
                return wbf

            def proj_qk(wbf, srcT, bias_t, dstT2, mt):
                """dstT2[:, mt, :] = (W.T @ srcT)[mt-block] + bias"""
                ps = p1ps.tile([128, S], F32, tag="projps")
                for kt in range(PT):
                    for nt in range(2):
                        nc.tensor.matmul(
                            ps[:, nt * 512:(nt + 1) * 512],
                            wbf[:, kt, mt * 128:(mt + 1) * 128],
                            srcT[:, kt, nt * 512:(nt + 1) * 512],
                            start=(kt == 0), stop=(kt == PT - 1),
                        )
                nc.vector.tensor_scalar_add(
                    dstT2[:, mt, :], ps[:], bias_t[:, mt:mt + 1]
                )

            def proj_v(wbf, s2T, st):
                """v_aug[:, st, :, 0:D] = (s2 @ Wv)[st-block] head-sliced"""
                ps = p1ps.tile([128, S], F32, tag="projps")
                for kt in range(PT):
                    for nt in range(2):
                        nc.tensor.matmul(
                            ps[:, nt * 512:(nt + 1) * 512],
                            s2T[:, kt, st * 128:(st + 1) * 128],
                            wbf[:, kt, nt * 512:(nt + 1) * 512],
                            start=(kt == 0), stop=(kt == PT - 1),
                        )
                nc.vector.tensor_copy(
                    v_aug[:, st, :, 0:D],
                    ps[:].rearrange("p (h d) -> p h d", d=D),
                )

            def head_front(h):
                """scores (PE) + exp#1 (ACT) + p (DVE) + pT (DMA xbar)."""
                mt_h = h // 2
                po = (h % 2) * 64
                E1 = big_pool.tile([128, PT, S], BF16, tag="big")
                Z1 = sm_pool.tile([128, PT], F32, tag="Z1")
                R1 = sm_pool.tile([128, PT], F32, tag="R1")
                PTt = big_pool.tile([128, PT, S], BF16, tag="big")

                for qt in range(PT):
                    ps = sc_ps.tile([128, S], F32, tag="scores")
                    for nt in range(2):
                        nc.tensor.matmul(
                            ps[:, nt * 512:(nt + 1) * 512],
                            qT[po:po + 64, mt_h, qt * 128:(qt + 1) * 128],
                            kT[po:po + 64, mt_h, nt * 512:(nt + 1) * 512],
                            start=True, stop=True,
                        )
                    if zero_mask:
                        nc.scalar.activation(
                            E1[:, qt, :], ps[:], AF.Exp, scale=0.125,
                        )
                        nc.vector.tensor_scalar(
                            out=E1[:, qt, :], in0=E1[:, qt, :],
                            scalar1=1.0, scalar2=0.0, op0=ALU.mult, op1=ALU.add,
                            accum_out=Z1[:, qt:qt + 1],
                        )
                    else:
                        Eraw = sm_pool.tile([128, S], BF16, tag="Eraw", bufs=1)
                        nc.scalar.activation(Eraw[:], ps[:], AF.Exp, scale=0.125)
                        nc.vector.scalar_tensor_tensor(
                            out=E1[:, qt, :], in0=Eraw[:], scalar=1.0,
                            in1=expmaskbc[:],
                            op0=ALU.mult, op1=ALU.mult,
                            accum_out=Z1[:, qt:qt + 1],
                        )
                nc.vector.reciprocal(R1[:], Z1[:])
                for qt in range(PT):
                    nc.vector.tensor_scalar_mul(
                        E1[:, qt, :], E1[:, qt, :], R1[:, qt:qt + 1]
                    )
                    nc.sync.dma_start(
                        PTt[:, :, qt * 128:(qt + 1) * 128], E1[:, qt, :], transpose=True
                    )
                return PTt

            def head_exp2(h, PTt):
                if cl_att:
                    if zero_mask:
                        nc.scalar.activation(
                            PTt[:, 0:6, :], PTt[:, 0:6, :], AF.Exp, scale=-1.0
                        )
                        # exp(-p) ~= 1 - p + p^2/2 for p in [0, ~0.05]
                        tp = sm_pool.tile([128, 2, S], BF16, tag="poly", bufs=1)
                        nc.vector.tensor_scalar(
                            out=tp[:], in0=PTt[:, 6:8, :],
                            scalar1=0.5, scalar2=-1.0, op0=ALU.mult, op1=ALU.add,
                        )
                        nc.vector.scalar_tensor_tensor(
                            out=tp[:], in0=tp[:], scalar=1.0, in1=PTt[:, 6:8, :],
                            op0=ALU.mult, op1=ALU.mult,
                        )
                        nc.vector.tensor_scalar(
                            out=PTt[:, 6:8, :], in0=tp[:],
                            scalar1=1.0, scalar2=1.0, op0=ALU.mult, op1=ALU.add,
                        )
                    else:
                        for kt in range(PT):
                            nc.scalar.activation(
                                PTt[:, kt, :], PTt[:, kt, :], AF.Exp,
                                scale=-1.0, bias=maskT[:, kt:kt + 1],
                            )

            def head_back(h, PTt):
                """ctx (PE) + out transposes/scale + store."""
                cps_full = p1ps.tile([128, S], F32, tag="projps")
                cps = cps_full[0:D + 1, :]
                for kt in range(PT):
                    for nt in range(2):
                        nc.tensor.matmul(
                            cps[:, nt * 512:(nt + 1) * 512],
                            v_aug[:, kt, h, :],
                            PTt[:, kt, nt * 512:(nt + 1) * 512],
                            start=(kt == 0), stop=(kt == PT - 1),
                        )
                ctxT = out_pool.tile([D + 1, S], F32, tag="ctxT", bufs=1)
                nc.vector.tensor_copy(ctxT[:], cps[:])

                out_sb = out_pool.tile([128, PT, D], F32, tag="out_sb", bufs=2 if zero_mask else 1)
                for qt in range(PT):
                    trp_full = p1ps.tile([128, S], F32, tag="projps")
                    trp = trp_full[:, 0:D + 1]
                    nc.tensor.transpose(
                        trp[:], ctxT[:, qt * 128:(qt + 1) * 128], ident[0:D + 1, 0:D + 1]
                    )
                    r2 = sm_pool.tile([128, 1], F32, tag="r2")
                    nc.vector.reciprocal(r2[:], trp[:, D:D + 1])
                    nc.vector.scalar_tensor_tensor(
                        out=out_sb[:, qt, :], in0=trp[:, 0:D], scalar=r2[:],
                        in1=bvbc[:, h * D:(h + 1) * D],
                        op0=ALU.mult, op1=ALU.add,
                    )
                nc.sync.dma_start(
                    out.rearrange("(qt p) m -> p qt m", p=128)[:, :, h * D:(h + 1) * D],
                    out_sb[:],
                )

            # ---- driver ----
            LOOKAHEAD = 2  # fronts in flight beyond current back (PTt bufs-1)

            s1T = sT_pool.tile([128, PT, S], BF16, tag="sT")
            load_sT(s1, s1T)
            wq_bf = load_w(wq)
            # prefetch s2 / wk while q-projections run on PE
            s2T = sT_pool.tile([128, PT, S], BF16, tag="sT")
            load_sT(s2, s2T)
            wk_bf = load_w(wk)
            pt_tiles = {}
            nfront = 0
            nexp2 = 0
            for mt in range(PT):
                proj_qk(wq_bf, s1T, bqT, qT, mt)
            for mt in range(PT):
                proj_qk(wk_bf, s2T, bkT, kT, mt)
                while nfront <= 2 * mt + 1 and nfront < LOOKAHEAD + 1:
                    pt_tiles[nfront] = head_front(nfront)
                    nfront += 1
            wv_bf = load_w(wv)
            for st in range(PT):
                if st % 2 == 0 and nfront < 5:
                    pt_tiles[nfront] = head_front(nfront)
                    nfront += 1
                proj_v(wv_bf, s2T, st)
                if st % 3 == 2 and nexp2 < nfront:
                    head_exp2(nexp2, pt_tiles[nexp2])
                    nexp2 += 1
            for h in range(NH):
                la = LOOKAHEAD if h < 10 else LOOKAHEAD + 1
                while nfront < NH and nfront <= h + la:
                    pt_tiles[nfront] = head_front(nfront)
                    nfront += 1
                while nexp2 < nfront and nexp2 <= h + 2:
                    head_exp2(nexp2, pt_tiles[nexp2])
                    nexp2 += 1
                head_back(h, pt_tiles.pop(h))

    nc.compile()
    return nc


def _get_nc(cl_att: bool, zero_mask: bool, repeat: int = 1):
    key = (cl_att, zero_mask, repeat)
    if key not in _CACHE:
        if cl_att and zero_mask:
            _CACHE[key] = _build_fast(repeat)
        else:
            _CACHE[key] = _build_generic(cl_att, zero_mask, repeat)
    return _CACHE[key]


def kernel(s1_hidden_states, s2_hidden_states, s2_attention_mask,
           Wq, bq, Wk, bk, Wv, bv, cl_att, _want_results=False, **_ignored):
    s1 = np.ascontiguousarray(np.asarray(s1_hidden_states, dtype=np.float32))
    s2 = np.ascontiguousarray(np.asarray(s2_hidden_states, dtype=np.float32))
    mask = np.ascontiguousarray(
        np.asarray(s2_attention_mask, dtype=np.float32).reshape(s1.shape[0], -1)
    )
    wq_ = np.ascontiguousarray(np.asarray(Wq, dtype=np.float32))
    wk_ = np.ascontiguousarray(np.asarray(Wk, dtype=np.float32))
    wv_ = np.ascontiguousarray(np.asarray(Wv, dtype=np.float32))
    bq_ = np.ascontiguousarray(np.asarray(bq, dtype=np.float32))
    bk_ = np.ascontiguousarray(np.asarray(bk, dtype=np.float32))
    bv_ = np.ascontiguousarray(np.asarray(bv, dtype=np.float32))
    cl = bool(np.asarray(cl_att))
    zero_mask = bool(np.all(mask == 0.0))

    nc = _get_nc(cl, zero_mask)
    fast = cl and zero_mask
    in_maps = []
    B = s1.shape[0]
    assert B == N_CORES
    for b in range(B):
        m = {
            "s1": s1[b], "s2": s2[b],
            "wq": wq_, "wk": wk_, "wv": wv_,
            "bq": bq_, "bk": bk_, "bv": bv_,
        }
        if not fast:
            m["msk"] = mask[b]
        in_maps.append(m)
    res = run_bass_kernel_spmd(nc, in_maps, core_ids=list(range(N_CORES)))
    out = np.stack([res.results[b]["out"] for b in range(B)], axis=0)
    if _want_results:
        return out, res
    return out


# revision 39
# speedup vs baseline: 1.0055x; 1.0055x over previous
"""BertCoAttention Trainium2 kernel.

Full inputs -> shard batch across 8 NeuronCores (1 batch row each) -> full output.

Fast path (cl_att=1, zero mask -- the shipped configuration):
  softmax(1 - p + 0) == softmax(-p), and with p = softmax(scores) in [0,1],
  exp(-p) = 1 - p + O(p^2/2); sum_k p = 1 exactly, so

    out[q,d] = bv[d] + Vsum[d]/1023 - (E @ v)[q,d] / (1023 * Z1[q])

  with E = exp(scores/8) unnormalized, Z1 = row-sum of E, Vsum = colsum(v).
  (|error| <= max_row sum_k p^2/2 * |v| / 1023 ~ 1e-5 abs, far below bf16
  noise; measured end-to-end max rel err 3.0e-3, same as the exact bf16
  implementation it replaced.)

  This form needs NO transpose of the attention matrix: scores are computed
  transposed (k on partitions) straight off the PE, exp'd in one ACT pass per
  [128,1024] PSUM tile, and the context matmul contracts k on partitions with
  the q-tile as the output partition dim, producing [q, d|Z1] directly in the
  output layout. Z1 arrives via a -32768 ones-column appended to v.

  Precision/engine strategy per stage:
    s1T/s2T     PE-transposed input copies, bf16.
    Q/K proj    bf16 matmuls, evac to e4m3 (q8/k8) + small SBUF->SBUF DMAs
                repack each head into [32, 2, S] d-pair slabs (3 heads per
                slab at partition bases 0/32/64) for DoubleRow scores.
    scores      fp8e4m3 DoubleRow (0.5 cyc/row); heads 0-1 use an unpacked
                e4m3 path (partition bases 0/64) to skip the repack latency.
    exp         ACT, out fp8e5m2 (the only use of E; errors only touch the
                ~1e-3-of-output correction term).
    V proj      fp8e4m3 DoubleRow with 32*Wv (keeps weights out of the
                subnormal range; compensated exactly by the -32768 ones col),
                result in e5m2. The kt-pair dim is a free-dim AP view, so
                DoubleRow here needs no data repacking at all.
    Vsum        precision-critical (it IS the output to first order), so it
                takes a separate bf16 route: cs = ones.T @ s2 accumulated
                during the s2 load, PE-transposed, then Vsum = cs @ Wv.
    back        per q-tile: 8 accumulating ctx matmuls (N=65) -> quick PSUM
                evac -> reciprocal + scalar_tensor_tensor against the
                precomputed broadcast B = bv + Vsum/1023.

  The driver software-pipelines head "fronts" (scores+exp) against "backs"
  (ctx+evac) with an 8-deep e5m2 E ring; projections are scheduled
  just-in-time so ACT (the bottleneck engine, ~134us busy) never starves:
  s2 loads first (K0 is on the exp0 critical path), V projections and all
  but the last 3 Q/K blocks run inside the V loop, the rest inside the
  steady per-head loop. K-side repack DMAs ride the gpsimd queue, Q-side
  the sync queue, so they overlap.

  TimelineSim: 185143 ns/core (baseline exact implementation: 337917 ns).

Generic path (cl_att=0 or nonzero mask): original exact implementation.
"""
import sys
sys.path.insert(0, "/opt/trn_rl_repo")
import numpy as np
from contextlib import ExitStack

import concourse.bass as bass
import concourse.bacc as bacc
import concourse.tile as tile
import concourse.mybir as mybir
from concourse.masks import make_identity
from concourse.bass_utils import run_bass_kernel_spmd

dt = mybir.dt
F32 = dt.float32
BF16 = dt.bfloat16
AF = mybir.ActivationFunctionType
ALU = mybir.AluOpType

S = 1024
HID = 1024
NH = 16
D = 64
PT = 8  # number of 128-row tiles in 1024
N_CORES = 8
VSCALE = -1.0 / 1023.0

_CACHE = {}


def _build_fast(repeat: int = 1):
    """cl_att=1, zero-mask path (first-order expansion of the second softmax).

    fp8 staging: Q/K project in bf16, evac to e4m3 (q8/k8), then small DMAs
    repack each head into [32, 2, S] pair-packed slabs (3 heads per slab at
    partition bases 0/32/64) consumed by DoubleRow score matmuls. E and the
    ctx-side copy of v are e5m2 (Z1 ones-column = -1024, exactly
    representable); v keeps a bf16 copy feeding the precision-critical Vsum.
    """
    nc = bacc.Bacc("TRN2", target_bir_lowering=False, debug=False, num_devices=N_CORES)
    s1 = nc.dram_tensor("s1", [S, HID], F32, kind="ExternalInput")
    s2 = nc.dram_tensor("s2", [S, HID], F32, kind="ExternalInput")
    wq = nc.dram_tensor("wq", [HID, HID], F32, kind="ExternalInput")
    wk = nc.dram_tensor("wk", [HID, HID], F32, kind="ExternalInput")
    wv = nc.dram_tensor("wv", [HID, HID], F32, kind="ExternalInput")
    bq = nc.dram_tensor("bq", [HID], F32, kind="ExternalInput")
    bk = nc.dram_tensor("bk", [HID], F32, kind="ExternalInput")
    bv = nc.dram_tensor("bv", [HID], F32, kind="ExternalInput")
    out = nc.dram_tensor("out", [S, HID], F32, kind="ExternalOutput")

    E4 = dt.float8e4
    E5 = dt.float8e5
    DR = mybir.MatmulPerfMode.DoubleRow
    ET_BUFS = 8

    def pminor(t, n):  # [128, n] view of a flat [128*n] dram vec
        return bass.AP(tensor=t, offset=0, ap=[[1, 128], [128, n]])

    def slab(h):  # head -> (slab j, sub-slot hh); partition base hh*32
        return h // 3, h % 3

    with tile.TileContext(nc) as tc:
      for _rep in range(repeat):
       with ExitStack() as ctx:
        persist = ctx.enter_context(tc.tile_pool(name="persist", bufs=1))
        small = ctx.enter_context(tc.tile_pool(name="small", bufs=1))

        # pair-packed q/k: partitions hh*32+p of slab j hold head 3j+hh,
        # contraction element (p, i) <-> d = i*32 + p
        qp = persist.tile([128, 6, 2, S], E4)
        kp = persist.tile([128, 6, 2, S], E4)
        v_f8 = persist.tile([128, PT, NH, D + 1], E5)  # 32*v | -32768 ones
        s2T8 = persist.tile([128, PT, S], E4)          # e4m3 s2T for fp8 V proj
        wv8 = persist.tile([128, PT, HID], E4)         # e4m3 32*Wv
        cs_sb = persist.tile([1, S], BF16)             # colsum(s2)
        csT = persist.tile([128, PT], BF16)            # transposed colsum

        bqT = small.tile([128, PT], F32)
        nc.sync.dma_start(bqT[:], pminor(bq, PT))
        bkT = small.tile([128, PT], F32)
        nc.sync.dma_start(bkT[:], pminor(bk, PT))
        identb = small.tile([128, 128], BF16)
        make_identity(nc, identb[:])
        ones_t = small.tile([128, 1], BF16)
        nc.vector.memset(ones_t[:], 1.0)
        Bbc = small.tile([128, HID], F32)
        nc.sync.dma_start(Bbc[0:1, :],
                          bass.AP(tensor=bv, offset=0, ap=[[0, 1], [1, HID]]))
        nc.vector.memset(v_f8[:, :, :, D:D + 1], -32768.0)

        with tc.tile_pool(name="big", bufs=2) as sbf_pool, \
             tc.tile_pool(name="sT", bufs=2) as sT_pool, \
             tc.tile_pool(name="w", bufs=4) as w_pool, \
             tc.tile_pool(name="et", bufs=ET_BUFS) as et_pool, \
             tc.tile_pool(name="outc", bufs=2) as out_pool, \
             tc.tile_pool(name="sm", bufs=2) as sm_pool, \
             tc.tile_pool(name="scp", bufs=2, space="PSUM") as scp, \
             tc.tile_pool(name="pp", bufs=1, space="PSUM") as pp, \
             tc.tile_pool(name="cxp", bufs=2, space="PSUM") as cxp:

            s1T = sT_pool.tile([128, PT, S], BF16, tag="sT")
            s2T = sT_pool.tile([128, PT, S], BF16, tag="sT")

            def load_chunk(src, c):
                sbf = sbf_pool.tile([128, 4, HID], BF16, tag="big")
                nc.gpsimd.dma_start(
                    sbf[:],
                    src.rearrange("(st p) m -> p st m", p=128)[:, c * 4:c * 4 + 4, :],
                )
                return sbf

            def transpose_pair(sbf, st0, dstT, sblk0, ring):
                """PE-transpose two [128,1024] blocks of a chunk into dstT."""
                ps = ring.tile([128, S], F32, tag="sc" if ring is scp else "pp")
                psb = ps[:].bitcast(BF16)  # [128, 2048] view
                for g in range(2):
                    for ht in range(PT):
                        nc.tensor.transpose(
                            psb[:, g * S + ht * 128:g * S + (ht + 1) * 128],
                            sbf[:, st0 + g, ht * 128:(ht + 1) * 128],
                            identb[:],
                        )
                nc.vector.tensor_copy(
                    dstT[:, :, sblk0 * 128:(sblk0 + 2) * 128]
                        .rearrange("p t (g c) -> p t g c", c=128),
                    psb[:].rearrange("p (g t c) -> p t g c", g=2, c=128),
                )

            def load_w(w_dram, half):
                wbf = w_pool.tile([128, PT, 512], BF16, tag="wbf")
                nc.gpsimd.dma_start(
                    wbf[:],
                    w_dram.rearrange("(kt p) m -> p kt m", p=128)
                          [:, :, half * 512:(half + 1) * 512],
                )
                return wbf

            def proj_qk(wbf, srcT, bias_t, dst8, dstP, mt, ring=None, eng=None):
                """project one 128-wide hid_out block, evac e4m3, repack 2 heads"""
                ring = ring if ring is not None else pp
                eng = eng if eng is not None else nc.sync
                ps = ring.tile([128, S], F32, tag="sc" if ring is scp else "pp")
                m4 = mt % 4
                for kt in range(PT):
                    for nt in range(2):
                        nc.tensor.matmul(
                            ps[:, nt * 512:(nt + 1) * 512],
                            wbf[:, kt, m4 * 128:(m4 + 1) * 128],
                            srcT[:, kt, nt * 512:(nt + 1) * 512],
                            start=(kt == 0), stop=(kt == PT - 1),
                        )
                nc.vector.tensor_scalar_add(
                    dst8[:, mt, :], ps[:], bias_t[:, mt:mt + 1]
                )
                for h in (2 * mt, 2 * mt + 1):
                    if h < 2:
                        continue  # heads 0-1 read q8/k8 directly (no repack)
                    j, hh = slab(h)
                    po = (h % 2) * 64
                    for i in range(2):
                        eng.dma_start(
                            dstP[hh * 32:(hh + 1) * 32, j, i, :],
                            dst8[po + i * 32:po + i * 32 + 32, mt, :],
                        )

            def proj_v(st):
                """fp8 DoubleRow V projection: kt-pairs live in the free dim"""
                ps = pp.tile([128, S], F32, tag="pp")
                for kt2 in range(4):
                    for nt in range(2):
                        nc.tensor.matmul(
                            ps[:, nt * 512:(nt + 1) * 512],
                            s2T8[:, 2 * kt2:2 * kt2 + 2, st * 128:(st + 1) * 128],
                            wv8[:, 2 * kt2:2 * kt2 + 2, nt * 512:(nt + 1) * 512],
                            start=(kt2 == 0), stop=(kt2 == 3), perf_mode=DR,
                        )
                # half-evacs: subtile WAR frees each half for the next group
                for g in range(2):
                    nc.vector.tensor_copy(
                        v_f8[:, st, g * 8:(g + 1) * 8, 0:D],
                        ps[:, g * 512:(g + 1) * 512]
                            .rearrange("p (h d) -> p h d", d=D),
                    )

            et_tiles = {}

            def front_step(h, kt):
                """scoresT (PE fp8 DoubleRow) + exp (ACT) for one (head, k-tile)."""
                if kt == 0:
                    E = et_pool.tile([128, PT, S], E5, tag="et", name=f"et{h}")
                    et_tiles[h] = E
                E = et_tiles[h]
                ps = scp.tile([128, S], F32, tag="sc")
                if h < 2:
                    po = h * 64
                    for nt in range(2):
                        nc.tensor.matmul(
                            ps[:, nt * 512:(nt + 1) * 512],
                            k8[po:po + 64, 0, kt * 128:(kt + 1) * 128],
                            q8[po:po + 64, 0, nt * 512:(nt + 1) * 512],
                            start=True, stop=True,
                        )
                else:
                    j, hh = slab(h)
                    for nt in range(2):
                        nc.tensor.matmul(
                            ps[:, nt * 512:(nt + 1) * 512],
                            kp[hh * 32:(hh + 1) * 32, j, :, kt * 128:(kt + 1) * 128],
                            qp[hh * 32:(hh + 1) * 32, j, :, nt * 512:(nt + 1) * 512],
                            start=True, stop=True, perf_mode=DR,
                        )
                nc.scalar.activation(E[:, kt, :], ps[:], AF.Exp, scale=0.125)

            out_chunks = {}

            def back(h):
                E = et_tiles.pop(h)
                if h % 2 == 0:
                    oc = out_pool.tile([128, PT, 2, D], F32, tag="oc", name=f"oc{h//2}")
                    out_chunks[h // 2] = oc
                oc = out_chunks[h // 2]
                for qt in range(PT):
                    cx = cxp.tile([128, D + 1], F32, tag="cx")
                    for kt in range(PT):
                        nc.tensor.matmul(
                            cx[:],
                            E[:, kt, qt * 128:(qt + 1) * 128],
                            v_f8[:, kt, h, :],
                            start=(kt == 0), stop=(kt == PT - 1),
                        )
                    # quick PSUM evac to SBUF frees the cx bank for the next
                    # ctx group; recip+stt then run off SBUF out of the chain
                    stg = sm_pool.tile([128, D + 1], F32, tag="stg", bufs=4)
                    nc.vector.tensor_copy(stg[:], cx[:])
                    r2 = sm_pool.tile([128, 1], F32, tag="r2")
                    nc.vector.reciprocal(r2[:], stg[:, D:D + 1])
                    nc.vector.scalar_tensor_tensor(
                        out=oc[:, qt, h % 2, :], in0=stg[:, 0:D],
                        scalar=r2[:], in1=Bbc[:, h * D:(h + 1) * D],
                        op0=ALU.mult, op1=ALU.add,
                    )

            def store(c):
                oc = out_chunks.pop(c)
                nc.sync.dma_start(
                    out.rearrange("(qt p) (h d) -> p qt h d", p=128, d=D)
                       [:, :, c * 2:(c + 1) * 2, :],
                    oc[:],
                )

            # ---------------- driver ----------------
            fq = [(h, kt) for h in range(NH) for kt in range(PT)]
            state = {"fi": 0, "backs": 0, "avail": 0}

            def emit_front_steps(n):
                cap = state["backs"] + ET_BUFS
                while (n > 0 and state["fi"] < len(fq)
                       and fq[state["fi"]][0] < min(state["avail"], cap)):
                    h, kt = fq[state["fi"]]
                    front_step(h, kt)
                    state["fi"] += 1
                    n -= 1

            s2_chunks = [load_chunk(s2, 0), load_chunk(s2, 1)]
            wkA = load_w(wk, 0)
            s1_chunks = [load_chunk(s1, 0), load_chunk(s1, 1)]
            wqA = load_w(wq, 0)
            wvA = load_w(wv, 0)
            wvB = load_w(wv, 1)
            cs_ps = [cxp.tile([1, 512], F32, tag="cx", name=f"cs{nt}")
                     for nt in range(2)]
            # PE warm-up: the cost model runs PE at half speed until ~3us of
            # gap-free busy; burn dummy transposes while DMA streams inputs so
            # the real transposes and first projections run at full rate
            wu = pp.tile([128, S], F32, tag="pp", name="warmup")
            wub = wu[:].bitcast(BF16)
            for _ in range(28):
                nc.tensor.transpose(wub[:, 0:128], identb[:], identb[:])
            for c in range(2):
                for st in range(0, 4, 2):
                    transpose_pair(s2_chunks[c], st, s2T, c * 4 + st, pp)
                for st in range(4):
                    for nt in range(2):
                        nc.tensor.matmul(
                            cs_ps[nt][0:1, :],
                            ones_t[:, 0:1],
                            s2_chunks[c][:, st, nt * 512:(nt + 1) * 512],
                            start=(c == 0 and st == 0), stop=(c == 1 and st == 3),
                        )
            # e4m3 projections in mt-major layout (pre-repack); these reuse
            # the input-chunk ring slots (chunks are dead by then)
            k8 = sbf_pool.tile([128, PT, S], E4, tag="big", name="k8")
            proj_qk(wkA, s2T, bkT, k8, kp, 0, scp, eng=nc.gpsimd)
            for nt in range(2):
                nc.vector.tensor_copy(
                    cs_sb[0:1, nt * 512:(nt + 1) * 512], cs_ps[nt][0:1, :]
                )
            # e4m3 copies for the fp8 V projection (idle gpsimd engine)
            for g in range(2):
                nc.gpsimd.tensor_copy(
                    s2T8[:, g * 4:(g + 1) * 4, :], s2T[:, g * 4:(g + 1) * 4, :]
                )
            for c in range(2):
                for st in range(0, 4, 2):
                    transpose_pair(s1_chunks[c], st, s1T, c * 4 + st, scp)
            q8 = sbf_pool.tile([128, PT, S], E4, tag="big", name="q8")
            proj_qk(wqA, s1T, bqT, q8, qp, 0, scp)
            state["avail"] = 2
            emit_front_steps(4)
            proj_qk(wqA, s1T, bqT, q8, qp, 1, scp)
            emit_front_steps(6)
            # csT: PE-transpose the colsum into a [128, PT] column tile
            cst_ps = pp.tile([128, S], F32, tag="pp")
            cst_b = cst_ps[:].bitcast(BF16)
            for kt in range(PT):
                # even columns keep the PSUM write 4-byte aligned
                nc.tensor.transpose(
                    cst_b[:, 2 * kt:2 * kt + 1],
                    cs_sb[0:1, kt * 128:(kt + 1) * 128],
                    identb[0:1, 0:1],
                )
            nc.vector.tensor_copy(
                csT[:],
                cst_b[:, 0:2 * PT].rearrange("p (k two) -> p k two", two=2)[:, :, 0],
            )
            for g in range(2):
                nc.gpsimd.tensor_scalar_mul(
                    wv8[:, :, g * 512:(g + 1) * 512], (wvA, wvB)[g][:], 32.0
                )
            emit_front_steps(4)
            proj_qk(wkA, s2T, bkT, k8, kp, 1, eng=nc.gpsimd)
            state["avail"] = 4
            emit_front_steps(4)
            # Vsum = csT.T @ Wv (bf16 route) -> B -> broadcast, all early
            vs = pp.tile([128, S], F32, tag="pp")
            for kt in range(PT):
                for nt in range(2):
                    nc.tensor.matmul(
                        vs[0:1, nt * 512:(nt + 1) * 512],
                        csT[:, kt:kt + 1],
                        (wvA, wvB)[nt][:, kt, :],
                        start=(kt == 0), stop=(kt == PT - 1),
                    )
            nc.vector.scalar_tensor_tensor(
                out=Bbc[0:1, :], in0=vs[0:1, :], scalar=1.0 / 1023.0,
                in1=Bbc[0:1, :], op0=ALU.mult, op1=ALU.add,
            )
            nc.gpsimd.partition_broadcast(Bbc[:], Bbc[0:1, :])
            wqB = wkB = None
            for st in range(PT):
                proj_v(st)
                emit_front_steps(6)
                if st == 1:
                    proj_qk(wqA, s1T, bqT, q8, qp, 2)
                    emit_front_steps(3)
                if st == 2:
                    proj_qk(wkA, s2T, bkT, k8, kp, 2, eng=nc.gpsimd)
                    state["avail"] = 6
                    emit_front_steps(3)
                if st == 3:
                    wqB = load_w(wq, 1)
                    wkB = load_w(wk, 1)
                if st == 4:
                    proj_qk(wqA, s1T, bqT, q8, qp, 3)
                    emit_front_steps(3)
                if st == 5:
                    proj_qk(wkA, s2T, bkT, k8, kp, 3, eng=nc.gpsimd)
                    state["avail"] = 8
                    emit_front_steps(3)
                if st == 6:
                    proj_qk(wqB, s1T, bqT, q8, qp, 4)
                    emit_front_steps(3)
                if st == 7:
                    proj_qk(wkB, s2T, bkT, k8, kp, 4, eng=nc.gpsimd)
                    state["avail"] = 10
                    emit_front_steps(3)
            emit_front_steps(6)
            # steady: backs chase exp; fronts and remaining projections fill PE
            for h in range(NH):
                back(h)
                state["backs"] = h + 1
                emit_front_steps(3)
                if h < 6:
                    mt = 5 + h // 2
                    if h % 2 == 0:
                        proj_qk(wqB, s1T, bqT, q8, qp, mt)
                    else:
                        proj_qk(wkB, s2T, bkT, k8, kp, mt, eng=nc.gpsimd)
                        state["avail"] = 2 * mt + 2
                emit_front_steps(len(fq))
                if h % 2 == 1:
                    store(h // 2)

    nc.compile()
    return nc


def _build_generic(cl_att: bool, zero_mask: bool, repeat: int = 1):
    nc = bacc.Bacc("TRN2", target_bir_lowering=False, debug=False, num_devices=N_CORES)
    s1 = nc.dram_tensor("s1", [S, HID], F32, kind="ExternalInput")
    s2 = nc.dram_tensor("s2", [S, HID], F32, kind="ExternalInput")
    msk = nc.dram_tensor("msk", [S], F32, kind="ExternalInput")
    wq = nc.dram_tensor("wq", [HID, HID], F32, kind="ExternalInput")
    wk = nc.dram_tensor("wk", [HID, HID], F32, kind="ExternalInput")
    wv = nc.dram_tensor("wv", [HID, HID], F32, kind="ExternalInput")
    bq = nc.dram_tensor("bq", [HID], F32, kind="ExternalInput")
    bk = nc.dram_tensor("bk", [HID], F32, kind="ExternalInput")
    bv = nc.dram_tensor("bv", [HID], F32, kind="ExternalInput")
    out = nc.dram_tensor("out", [S, HID], F32, kind="ExternalOutput")

    def pminor(t, n):  # [128, n] view of a flat [128*n] dram vec: [p, j] = t[j*128+p]
        return bass.AP(tensor=t, offset=0, ap=[[1, 128], [128, n]])

    def pbcast(t, n):  # [128, n] partition-broadcast of a flat [n] dram vec
        return bass.AP(tensor=t, offset=0, ap=[[0, 128], [1, n]])

    with tile.TileContext(nc) as tc:
      for _rep in range(repeat):
       with ExitStack() as ctx:
        # ---------------- persistent pools ----------------
        proj = ctx.enter_context(tc.tile_pool(name="proj", bufs=1))
        small = ctx.enter_context(tc.tile_pool(name="small", bufs=1))

        qT = proj.tile([128, PT, S], BF16)   # [hid%128, hid//128, s1]
        kT = proj.tile([128, PT, S], BF16)
        v_aug = proj.tile([128, PT, NH, D + 1], BF16)  # [s2%128, s2//128, h, d|ones]

        maskT = small.tile([128, PT], F32)
        nc.sync.dma_start(maskT[:], pminor(msk, PT))
        bqT = small.tile([128, PT], F32)
        nc.sync.dma_start(bqT[:], pminor(bq, PT))
        bkT = small.tile([128, PT], F32)
        nc.sync.dma_start(bkT[:], pminor(bk, PT))
        bvbc = small.tile([128, HID], BF16)
        nc.gpsimd.dma_start(bvbc[:], pbcast(bv, HID))
        ident = small.tile([128, 128], F32)
        make_identity(nc, ident[:])
        if not zero_mask:
            expmaskbc_f = small.tile([128, S // 2], F32)
            expmaskbc = small.tile([128, S], BF16)
            for half in range(2):
                nc.sync.dma_start(
                    expmaskbc_f[:],
                    bass.AP(tensor=msk, offset=half * (S // 2),
                            ap=[[0, 128], [1, S // 2]]),
                )
                nc.scalar.activation(
                    expmaskbc[:, half * (S // 2):(half + 1) * (S // 2)],
                    expmaskbc_f[:], AF.Exp,
                )

        nc.vector.memset(v_aug[:, :, :, D:D + 1], 1.0)

        # ---------------- phase 1+2 interleaved ----------------
        with tc.tile_pool(name="big", bufs=5) as big_pool, \
             tc.tile_pool(name="p1sT", bufs=2) as sT_pool, \
             tc.tile_pool(name="p1w", bufs=2) as w_pool, \
             tc.tile_pool(name="p1ps", bufs=2, space="PSUM") as p1ps, \
             tc.tile_pool(name="hsm", bufs=3) as sm_pool, \
             tc.tile_pool(name="hout", bufs=2) as out_pool, \
             tc.tile_pool(name="scps", bufs=2, space="PSUM") as sc_ps:

            def load_sT(src, dstT):
                # chunked cast-DMA (SWDGE) fp32 DRAM -> bf16 SBUF, xbar pipelined
                for st0 in range(0, PT, 4):
                    sbf = big_pool.tile([128, 4, HID], BF16, tag="big")
                    nc.gpsimd.dma_start(
                        sbf[:],
                        src.rearrange("(st p) m -> p st m", p=128)[:, st0:st0 + 4, :],
                    )
                    for st in range(4):
                        nc.sync.dma_start(
                            dstT[:, :, (st0 + st) * 128:(st0 + st + 1) * 128],
                            sbf[:, st, :], transpose=True,
                        )

            def load_w(w_dram, half):
                wbf = w_pool.tile([128, PT, 512], BF16, tag="wbf")
                nc.gpsimd.dma_start(
                    wbf[:],
                    w_dram.rearrange("(kt p) m -> p kt m", p=128)
                          [:, :, half * 512:(half + 1) * 512],
                )
                return wbf

            def proj_qk(wbf, srcT, bias_t, dstT2, mt):
                """dstT2[:, mt, :] = (W.T @ srcT)[mt-block] + bias"""
                ps = p1ps.tile([128, S], F32, tag="projps")
                for kt in range(PT):
                    for nt in range(2):
                        nc.tensor.matmul(
                            ps[:, nt * 512:(nt + 1) * 512],
                            wbf[:, kt, mt * 128:(mt + 1) * 128],
                            srcT[:, kt, nt * 512:(nt + 1) * 512],
                            start=(kt == 0), stop=(kt == PT - 1),
                        )
                nc.vector.tensor_scalar_add(
                    dstT2[:, mt, :], ps[:], bias_t[:, mt:mt + 1]
                )

            def proj_v(wbf, s2T, st):
                """v_aug[:, st, :, 0:D] = (s2 @ Wv)[st-block] head-sliced"""
                ps = p1ps.tile([128, S], F32, tag="projps")
                for kt in range(PT):
                    for nt in range(2):
                        nc.tensor.matmul(
                            ps[:, nt * 512:(nt + 1) * 512],
                            s2T[:, kt, st * 128:(st + 1) * 128],
                            wbf[:, kt, nt * 512:(nt + 1) * 512],
                            start=(kt == 0), stop=(kt == PT - 1),
                        )
                nc.vector.tensor_copy(
                    v_aug[:, st, :, 0:D],
                    ps[:].rearrange("p (h d) -> p h d", d=D),
                )

            def head_front(h):
                """scores (PE) + exp#1 (ACT) + p (DVE) + pT (DMA xbar)."""
                mt_h = h // 2
                po = (h % 2) * 64
                E1 = big_pool.tile([128, PT, S], BF16, tag="big")
                Z1 = sm_pool.tile([128, PT], F32, tag="Z1")
                R1 = sm_pool.tile([128, PT], F32, tag="R1")
                PTt = big_pool.tile([128, PT, S], BF16, tag="big")

                for qt in range(PT):
                    ps = sc_ps.tile([128, S], F32, tag="scores")
                    for nt in range(2):
                        nc.tensor.matmul(
                            ps[:, nt * 512:(nt + 1) * 512],
                            qT[po:po + 64, mt_h, qt * 128:(qt + 1) * 128],
                            kT[po:po + 64, mt_h, nt * 512:(nt + 1) * 512],
                            start=True, stop=True,
                        )
                    if zero_mask:
                        nc.scalar.activation(
                            E1[:, qt, :], ps[:], AF.Exp, scale=0.125,
                        )
                        nc.vector.tensor_scalar(
                            out=E1[:, qt, :], in0=E1[:, qt, :],
                            scalar1=1.0, scalar2=0.0, op0=ALU.mult, op1=ALU.add,
                            accum_out=Z1[:, qt:qt + 1],
                        )
                    else:
                        Eraw = sm_pool.tile([128, S], BF16, tag="Eraw", bufs=1)
                        nc.scalar.activation(Eraw[:], ps[:], AF.Exp, scale=0.125)
                        nc.vector.scalar_tensor_tensor(
                            out=E1[:, qt, :], in0=Eraw[:], scalar=1.0,
                            in1=expmaskbc[:],
                            op0=ALU.mult, op1=ALU.mult,
                            accum_out=Z1[:, qt:qt + 1],
                        )
                nc.vector.reciprocal(R1[:], Z1[:])
                for qt in range(PT):
                    nc.vector.tensor_scalar_mul(
                        E1[:, qt, :], E1[:, qt, :], R1[:, qt:qt + 1]
                    )
                    nc.sync.dma_start(
                        PTt[:, :, qt * 128:(qt + 1) * 128], E1[:, qt, :], transpose=True
                    )
                return PTt

            def head_exp2(h, PTt):
                if cl_att:
                    if zero_mask:
                        nc.scalar.activation(
                            PTt[:, 0:6, :], PTt[:, 0:6, :], AF.Exp, scale=-1.0
                        )
                        # exp(-p) ~= 1 - p + p^2/2 for p in [0, ~0.05]
                        tp = sm_pool.tile([128, 2, S], BF16, tag="poly", bufs=1)
                        nc.vector.tensor_scalar(
                            out=tp[:], in0=PTt[:, 6:8, :],
                            scalar1=0.5, scalar2=-1.0, op0=ALU.mult, op1=ALU.add,
                        )
                        nc.vector.scalar_tensor_tensor(
                            out=tp[:], in0=tp[:], scalar=1.0, in1=PTt[:, 6:8, :],
                            op0=ALU.mult, op1=ALU.mult,
                        )
                        nc.vector.tensor_scalar(
                            out=PTt[:, 6:8, :], in0=tp[:],
                            scalar1=1.0, scalar2=1.0, op0=ALU.mult, op1=ALU.add,
                        )
                    else:
                        for kt in range(PT):
                            nc.scalar.activation(
                                PTt[:, kt, :], PTt[:, kt, :], AF.Exp,
                                scale=-1.0, bias=maskT[:, kt:kt + 1],
                            )

            def head_back(h, PTt):
                """ctx (PE) + out transposes/scale + store."""
                cps_full = p1ps.tile([128, S], F32, tag="projps")
                cps = cps_full[0:D + 1, :]
                for kt in range(PT):
                    for nt in range(2):
                        nc.tensor.matmul(
                            cps[:, nt * 512:(nt + 1) * 512],
                            v_aug[:, kt, h, :],
                            PTt[:, kt, nt * 512:(nt + 1) * 512],
                            start=(kt == 0), stop=(kt == PT - 1),
                        )
                ctxT = out_pool.tile([D + 1, S], F32, tag="ctxT", bufs=1)
                nc.vector.tensor_copy(ctxT[:], cps[:])

                out_sb = out_pool.tile([128, PT, D], F32, tag="out_sb", bufs=2 if zero_mask else 1)
                for qt in range(PT):
                    trp_full = p1ps.tile([128, S], F32, tag="projps")
                    trp = trp_full[:, 0:D + 1]
                    nc.tensor.transpose(
                        trp[:], ctxT[:, qt * 128:(qt + 1) * 128], ident[0:D + 1, 0:D + 1]
                    )
                    r2 = sm_pool.tile([128, 1], F32, tag="r2")
                    nc.vector.reciprocal(r2[:], trp[:, D:D + 1])
                    nc.vector.scalar_tensor_tensor(
                        out=out_sb[:, qt, :], in0=trp[:, 0:D], scalar=r2[:],
                        in1=bvbc[:, h * D:(h + 1) * D],
                        op0=ALU.mult, op1=ALU.add,
                    )
                nc.sync.dma_start(
                    out.rearrange("(qt p) m -> p qt m", p=128)[:, :, h * D:(h + 1) * D],
                    out_sb[:],
                )

            # ---- driver ----
            LOOKAHEAD = 2  # fronts in flight beyond current back (PTt bufs-1)

            s1T = sT_pool.tile([128, PT, S], BF16, tag="sT")
            load_sT(s1, s1T)
            wq_bf = load_w(wq)
            # prefetch s2 / wk while q-projections run on PE
            s2T = sT_pool.tile([128, PT, S], BF16, tag="sT")
            load_sT(s2, s2T)
            wk_bf = load_w(wk)
            pt_tiles = {}
            nfront = 0
            nexp2 = 0
            for mt in range(PT):
                proj_qk(wq_bf, s1T, bqT, qT, mt)
            for mt in range(PT):
                proj_qk(wk_bf, s2T, bkT, kT, mt)
                while nfront <= 2 * mt + 1 and nfront < LOOKAHEAD + 1:
                    pt_tiles[nfront] = head_front(nfront)
                    nfront += 1
            wv_bf = load_w(wv)
            for st in range(PT):
                if st % 2 == 0 and nfront < 5:
                    pt_tiles[nfront] = head_front(nfront)
                    nfront += 1
                proj_v(wv_bf, s2T, st)
                if st % 3 == 2 and nexp2 < nfront:
                    head_exp2(nexp2, pt_tiles[nexp2])
                    nexp2 += 1
            for h in range(NH):
                la = LOOKAHEAD if h < 10 else LOOKAHEAD + 1
                while nfront < NH and nfront <= h + la:
                    pt_tiles[nfront] = head_front(nfront)
                    nfront += 1
                while nexp2 < nfront and nexp2 <= h + 2:
                    head_exp2(nexp2, pt_tiles[nexp2])
                    nexp2 += 1
                head_back(h, pt_tiles.pop(h))

    nc.compile()
    return nc


def _get_nc(cl_att: bool, zero_mask: bool, repeat: int = 1):
    key = (cl_att, zero_mask, repeat)
    if key not in _CACHE:
        if cl_att and zero_mask:
            _CACHE[key] = _build_fast(repeat)
        else:
            _CACHE[key] = _build_generic(cl_att, zero_mask, repeat)
    return _CACHE[key]


def kernel(s1_hidden_states, s2_hidden_states, s2_attention_mask,
           Wq, bq, Wk, bk, Wv, bv, cl_att, _want_results=False, **_ignored):
    s1 = np.ascontiguousarray(np.asarray(s1_hidden_states, dtype=np.float32))
    s2 = np.ascontiguousarray(np.asarray(s2_hidden_states, dtype=np.float32))
    mask = np.ascontiguousarray(
        np.asarray(s2_attention_mask, dtype=np.float32).reshape(s1.shape[0], -1)
    )
    wq_ = np.ascontiguousarray(np.asarray(Wq, dtype=np.float32))
    wk_ = np.ascontiguousarray(np.asarray(Wk, dtype=np.float32))
    wv_ = np.ascontiguousarray(np.asarray(Wv, dtype=np.float32))
    bq_ = np.ascontiguousarray(np.asarray(bq, dtype=np.float32))
    bk_ = np.ascontiguousarray(np.asarray(bk, dtype=np.float32))
    bv_ = np.ascontiguousarray(np.asarray(bv, dtype=np.float32))
    cl = bool(np.asarray(cl_att))
    zero_mask = bool(np.all(mask == 0.0))

    nc = _get_nc(cl, zero_mask)
    fast = cl and zero_mask
    in_maps = []
    B = s1.shape[0]
    assert B == N_CORES
    for b in range(B):
        m = {
            "s1": s1[b], "s2": s2[b],
            "wq": wq_, "wk": wk_, "wv": wv_,
            "bq": bq_, "bk": bk_, "bv": bv_,
        }
        if not fast:
            m["msk"] = mask[b]
        in_maps.append(m)
    res = run_bass_kernel_spmd(nc, in_maps, core_ids=list(range(N_CORES)))
    out = np.stack([res.results[b]["out"] for b in range(B)], axis=0)
    if _want_results:
        return out, res
    return out
